# revision 8
# baseline (speedup 1.0000x reference)
"""Chamfer loss kernel for 8 Trainium2 NeuronCores.

Problem: x, y: [4, 8192, 3] f32. loss = sum_b [ sum_n min_m d(x_bn, y_bm)
+ sum_m min_n d(x_bn, y_bm) ].

Sharding: 8 cores = 4 batches x 2 directions. Core c handles batch c//2;
direction c%2 swaps (query, reference) roles, so every core computes one
full 8192x8192 distance-squared tile and its row minima. The scalar
reduction (sqrt + sum over the 8*8192 row minima) is done on host.

Device math: d2[n,m] = |q_n|^2 + |r_m|^2 - 2 q_n . r_m is computed on the
PE as a K=24 matmul of bf16 triple-split operands (near-fp32 precision at
bf16 speed), accumulated fp32 in PSUM. Row minima via tensor_tensor_scan
with op0=op1=min: state = min(state, psum_chunk[t], evac_chunk[t]) — one
DVE pass consumes two chunks (a PSUM chunk and a ScalarE-evacuated SBUF
copy of its sibling), chained across chunk-pairs via the scan's initial
value; the row minimum is the last element of the final scan output.
"""
import sys
import types

import numpy as np
import ml_dtypes

_BF16 = ml_dtypes.bfloat16

B, N, D = 4, 8192, 3
P = 128              # partition tile (rows per row-tile)
MMW = 512            # matmul moving width (one fp32 PSUM bank)
CH = 1024            # chunk width = 2 PSUM banks
K = 24               # contraction rows after decomposition
INF = float(np.float32(3.0e38))

_compiled = None


def _shim_axon_hooks():
    """bass_utils wants antenv.axon_hooks for NTFF tracing; this image
    lacks it. Provide it, backed by the ctypes hook from trn_agent_boot."""
    if 'antenv.axon_hooks' in sys.modules:
        return
    hook = None
    try:
        import antenv  # noqa: F401
        from trn_agent_boot.trn_boot import _ntff_profile_via_ctypes
        hook = _ntff_profile_via_ctypes('/opt/axon/libaxon_pjrt.so')
    except Exception:
        hook = None
    mod = types.ModuleType('antenv.axon_hooks')
    mod.get_axon_ntff_profile_hook = lambda: hook
    mod.set_axon_ntff_profile_hook = lambda h: None
    sys.modules['antenv.axon_hooks'] = mod


def _split3(a):
    """Triple bf16 split of fp32 array: a ~ s0+s1+s2 with ~2^-27 residual."""
    a = a.astype(np.float32)
    s0 = a.astype(_BF16)
    r = a - s0.astype(np.float32)
    s1 = r.astype(_BF16)
    r = r - s1.astype(np.float32)
    s2 = r.astype(_BF16)
    return s0, s1, s2


def _prep_core(q, r, n=None):
    """Build lhsT [24, n] bf16 (stationary/query side) and rhs [24, n] bf16
    (moving/reference side). Row order = PE accumulation order: the large
    |q|^2, |r|^2 terms first, then products in decreasing magnitude, so
    fp32 partial-sum rounding stays at the ~1e-7 level."""
    n = n or N
    q = q.astype(np.float32)
    w = (-2.0 * r).astype(np.float32)
    q0, q1, q2 = _split3(q)
    w0, w1, w2 = _split3(w)
    qq0, qq1, qq2 = _split3((q * q).sum(-1))
    rr0, rr1, rr2 = _split3((r.astype(np.float32) ** 2).sum(-1))

    ones = np.ones(n, dtype=_BF16)
    lhsT = np.empty((K, n), dtype=_BF16)
    rhs = np.empty((K, n), dtype=_BF16)
    lhsT[0], lhsT[1], lhsT[2] = qq0, qq1, qq2
    rhs[0] = rhs[1] = rhs[2] = ones
    lhsT[3] = lhsT[4] = lhsT[5] = ones
    rhs[3], rhs[4], rhs[5] = rr0, rr1, rr2
    pairs = [(q0, w0), (q0, w1), (q1, w0), (q1, w1), (q0, w2), (q2, w0)]
    for i, (qa, wb) in enumerate(pairs):
        base = 6 + 3 * i
        lhsT[base:base + 3] = qa.T
        rhs[base:base + 3] = wb.T
    return lhsT, rhs


def build_program(nc, n=None):
    """Emit the per-core program. n = number of points (8192 in prod)."""
    import concourse.tile as tile
    import concourse.mybir as mybir

    n = n or N
    nt = n // P
    npair = n // (2 * CH)
    lhsT = nc.dram_tensor("lhsT", [K, n], mybir.dt.bfloat16,
                          kind="ExternalInput").ap()
    rhs = nc.dram_tensor("rhs", [K, n], mybir.dt.bfloat16,
                         kind="ExternalInput").ap()
    out = nc.dram_tensor("out", [P, nt], mybir.dt.float32,
                         kind="ExternalOutput").ap()

    mn = mybir.AluOpType.min
    with tile.TileContext(nc) as tc:
        with tc.tile_pool(name="inp", bufs=1) as inp, \
             tc.tile_pool(name="accp", bufs=1) as accp, \
             tc.tile_pool(name="ps", bufs=4, space="PSUM") as psp, \
             tc.tile_pool(name="evac", bufs=3) as evacp, \
             tc.tile_pool(name="scan", bufs=3) as scanp:
            tl = inp.tile([K, n], mybir.dt.bfloat16)
            nc.sync.dma_start(tl[:], lhsT[:])
            tr = inp.tile([K, n], mybir.dt.bfloat16)
            nc.sync.dma_start(tr[:], rhs[:])
            acc = accp.tile([P, nt], mybir.dt.float32)

            for t in range(nt):
                lt = tl[:, t * P:(t + 1) * P]
                s_prev = None
                for pair in range(npair):
                    base = pair * (2 * CH)
                    cA = psp.tile([P, CH], mybir.dt.float32, tag="ps")
                    for j in range(CH // MMW):
                        nc.tensor.matmul(
                            cA[:, j * MMW:(j + 1) * MMW], lt,
                            tr[:, base + j * MMW: base + (j + 1) * MMW],
                            start=True, stop=True)
                    cB = psp.tile([P, CH], mybir.dt.float32, tag="ps")
                    for j in range(CH // MMW):
                        nc.tensor.matmul(
                            cB[:, j * MMW:(j + 1) * MMW], lt,
                            tr[:, base + CH + j * MMW:
                               base + CH + (j + 1) * MMW],
                            start=True, stop=True)
                    ev = evacp.tile([P, CH], mybir.dt.bfloat16)
                    nc.scalar.copy(ev[:], cB[:])
                    s = scanp.tile([P, CH], mybir.dt.float32)
                    nc.vector.tensor_tensor_scan(
                        s[:], cA[:], ev[:],
                        (INF if s_prev is None else s_prev[:, CH - 1:CH]),
                        mn, mn)
                    s_prev = s
                nc.scalar.copy(acc[:, t:t + 1], s_prev[:, CH - 1:CH])
            nc.sync.dma_start(out[:], acc[:])
    nc.compile()
    return nc


def _build_program():
    global _compiled
    if _compiled is not None:
        return _compiled
    _shim_axon_hooks()
    from concourse import bacc
    nc = bacc.Bacc("TRN2", target_bir_lowering=False, debug=False)
    build_program(nc)
    _compiled = nc
    return nc


def _run_cores(in_maps, trace=False):
    _shim_axon_hooks()
    from concourse import bass_utils
    nc = _build_program()
    return bass_utils.run_bass_kernel_spmd(
        nc, in_maps, core_ids=list(range(2 * B)), trace=trace)


def kernel(x, y, _trace=False, _return_results=False):
    x = np.asarray(x, dtype=np.float32)
    y = np.asarray(y, dtype=np.float32)
    in_maps = []
    for c in range(2 * B):
        b = c // 2
        q, r = (x[b], y[b]) if c % 2 == 0 else (y[b], x[b])
        lhsT, rhs = _prep_core(q, r)
        in_maps.append({"lhsT": lhsT, "rhs": rhs})

    res = _run_cores(in_maps, trace=_trace)

    total = 0.0
    for c in range(2 * B):
        d2 = res.results[c]["out"].T.reshape(N).astype(np.float64)
        total += np.sqrt(np.maximum(d2, 0.0)).sum()
    loss = np.asarray(np.float32(total))
    if _return_results:
        return loss, res
    return loss
